# revision 5
# baseline (speedup 1.0000x reference)
"""VQ-codebook 3x3 conv (nn_CConv) on 8 Trainium2 NeuronCores.

Sharding: data-parallel over the batch (16 images -> 2 per core); the small
codebook-derived weights / scales / bias are replicated to every core.
Host-side work is layout/dtype prep only: batch split, zero-padding of x to
114x114 with an f16 cast, reshape/transpose of the index and scale matrices,
the codebook row gather (pure indexing), and stripping the two junk columns
from the padded f16 output.

Per-core device program (one NEFF, SPMD over 8 cores):
  - weight build (on device): multiply the gathered codebook rows by
    scales (pre-rounded through f16 on host = dequant emulation) * cut;
    weights stored k-major so each of the 9 taps is a contiguous
    [128(in), 128(out)] fp16 stationary block.  Weight DMAs ride the
    gpsimd HWDGE queue so they overlap the first input-image load.
  - conv: each padded image lives whole in SBUF ([128, 114*114] f16,
    loaded in 4 contiguous chunked DMAs on the sync queue); the 3x3 conv
    is 9 accumulating PE matmuls over shifted views of the flattened
    padded image, fp16 in / fp32 PSUM accumulate, 25 PSUM tiles of
    <=512 positions per image.  Outputs are computed in padded q space
    (q = h*114 + w); junk columns w in {112,113} are kept in the f16
    DRAM output and stripped on host, so both input and output DMAs are
    fully contiguous per partition.
  - PSUM is evacuated with a fused per-partition bias add on the vector
    engine straight to f16; output DMAs ride the scalar HWDGE queue in
    row-chunks as soon as their tiles are evacuated.
  - 8 dummy warm-up matmuls run during the prologue so the PE HAM clock
    gate reaches 2.4 GHz before the real matmuls start.
"""
import sys
import types
from contextlib import ExitStack

import numpy as np

import concourse.tile as tile
from concourse import bacc, mybir


def _ensure_axon_hooks_module():
    """This image's antenv package lacks axon_hooks; bass_utils imports it
    when tracing is requested (e.g. BASS_TRACE=1). Provide a no-op shim."""
    try:
        import antenv

        if "antenv.axon_hooks" not in sys.modules and not hasattr(
            antenv, "axon_hooks"
        ):
            mod = types.ModuleType("antenv.axon_hooks")
            holder = [None]
            mod.set_axon_ntff_profile_hook = lambda h: holder.__setitem__(0, h)
            mod.get_axon_ntff_profile_hook = lambda: holder[0]
            antenv.axon_hooks = mod
            sys.modules["antenv.axon_hooks"] = mod
    except Exception:
        pass


_ensure_axon_hooks_module()

from concourse import bass_utils  # noqa: E402

P = 128
H = W = 112
WP = 114
NPIX = WP * WP          # padded image pixels per channel (114*114 = 12996)
NQ = H * WP             # output q-space length incl. junk cols (112*114 = 12768)
IMGS = 2
N_CORES = 8

f32 = mybir.dt.float32
f16 = mybir.dt.float16

WARM_MMS = 8
IN_CHUNKS = [(0, 30), (30, 58), (58, 86), (86, 114)]       # padded input rows
OUT_CHUNKS = [(0, 28), (28, 56), (56, 84), (84, 112)]      # output rows
OUT_CHUNKS_LAST = [(0, 28), (28, 56), (56, 84), (84, 106), (106, 112)]

_CACHE = {}


def _q_tiles():
    full, r = divmod(NQ, 512)
    return [512] * full + ([r] if r else [])


def _build():
    nc = bacc.Bacc("TRN2", target_bir_lowering=False, debug=False)

    x_t = nc.dram_tensor("x", [IMGS, P, NPIX], f16, kind="ExternalInput")
    bias_t = nc.dram_tensor("bias", [P, 1], f32, kind="ExternalInput")
    wmm_t = nc.dram_tensor("wmm", [P, P * 9], f16, kind="ExternalInput")
    out_t = nc.dram_tensor("out", [IMGS, P, NQ], f16, kind="ExternalOutput")

    with tile.TileContext(nc) as tc, ExitStack() as ctx:
        wb = ctx.enter_context(tc.tile_pool(name="wb", bufs=1))
        xp = ctx.enter_context(tc.tile_pool(name="xp", bufs=2))
        op = ctx.enter_context(tc.tile_pool(name="op", bufs=2))
        ps = ctx.enter_context(tc.tile_pool(name="ps", bufs=6, space="PSUM"))

        # PE warmup: HAM un-throttles to 2.4 GHz during the prologue
        wrm = wb.tile([P, 512], f16, tag="warm")
        nc.gpsimd.memset(wrm[:], 0.0)
        pw = ps.tile([P, 512], f32, tag="pst")
        for _ in range(WARM_MMS):
            nc.tensor.matmul(pw[:], wrm[:, :P], wrm[:], start=True, stop=True)

        # ---- weights: host-built w_mm[i, k, o] (k-major f16 taps) ----
        w_mm = wb.tile([P, 9 * P], f16, tag="w_mm")
        nc.sync.dma_start(w_mm[:], wmm_t.ap())
        bias_s = wb.tile([P, 1], f32, tag="bias")
        nc.sync.dma_start(bias_s[:], bias_t.ap())
        w_k_view = w_mm[:].rearrange("p (k o) -> p k o", o=P)

        # ---- conv, whole image resident in SBUF ----
        for img in range(IMGS):
            xpad = xp.tile([P, NPIX + 2], f16, tag="xpad")
            # last 2 junk positions read 2 elements past the padded image
            nc.gpsimd.memset(xpad[:, NPIX:NPIX + 2], 0.0)
            for r0, r1 in IN_CHUNKS:
                nc.sync.dma_start(
                    xpad[:, r0 * WP:r1 * WP], x_t.ap()[img, :, r0 * WP:r1 * WP]
                )

            oimg = op.tile([P, NQ], f16, tag="oimg")
            ochunks = list(OUT_CHUNKS_LAST if img == IMGS - 1 else OUT_CHUNKS)
            q0 = 0
            for n in _q_tiles():
                pst = ps.tile([P, 512], f32, tag="pst")
                for k in range(9):
                    dh, dw = divmod(k, 3)
                    off = q0 + dh * WP + dw
                    nc.tensor.matmul(
                        pst[:, :n],
                        w_k_view[:, k, :],
                        xpad[:, off:off + n],
                        start=(k == 0),
                        stop=(k == 8),
                    )
                nc.vector.tensor_scalar_add(
                    oimg[:, q0:q0 + n], pst[:, :n], bias_s[:, 0:1]
                )
                q0 += n
                while ochunks and q0 >= ochunks[0][1] * WP:
                    r0, r1 = ochunks.pop(0)
                    nc.scalar.dma_start(
                        out_t.ap()[img, :, r0 * WP:r1 * WP],
                        oimg[:, r0 * WP:r1 * WP],
                    )
            assert not ochunks

    nc.compile()
    return nc


def _make_in_maps(inputs):
    x = np.asarray(inputs["x"], dtype=np.float32)
    nimg = x.shape[0]
    xpad = np.zeros((nimg, P, WP, WP), dtype=np.float16)
    xpad[:, :, 1:1 + H, 1:1 + W] = x.astype(np.float16)
    xpad = xpad.reshape(nimg, P, NPIX)

    cent = np.asarray(inputs["centroids"], dtype=np.float32).reshape(512, 9)
    idxT = np.asarray(inputs["idx"]).reshape(P, P).T          # [i, o]
    # fp16 round-trip of scales (dequant emulation), * cut
    scalesT = (
        np.asarray(inputs["scales"], dtype=np.float32).reshape(P, P).T
        .astype(np.float16).astype(np.float32)
    )
    cutT = np.asarray(inputs["cut"], dtype=np.float32).reshape(P, P).T
    bias = np.ascontiguousarray(
        np.asarray(inputs["bias"], dtype=np.float32).reshape(P, 1)
    )
    # w_mm[i, k, o] = w_raw[i, o, k] * scales_q[i, o] * cut[i, o], f16 taps
    wraw = cent[idxT].reshape(P, P, 9)                        # [i, o, k]
    wmm = np.ascontiguousarray(
        (wraw * (scalesT * cutT)[:, :, None])
        .transpose(0, 2, 1).reshape(P, P * 9).astype(np.float16)
    )

    base = {"bias": bias, "wmm": wmm}
    maps = []
    for c in range(N_CORES):
        m = dict(base)
        m["x"] = np.ascontiguousarray(xpad[IMGS * c:IMGS * (c + 1)])
        maps.append(m)
    return maps


def _get_nc():
    if "nc" not in _CACHE:
        _CACHE["nc"] = _build()
    return _CACHE["nc"]


def _run(inputs, trace=False):
    nc = _get_nc()
    in_maps = _make_in_maps(inputs)
    res = bass_utils.run_bass_kernel_spmd(
        nc, in_maps, core_ids=list(range(N_CORES)), trace=trace
    )
    outp = np.concatenate(
        [res.results[c]["out"] for c in range(N_CORES)], axis=0
    )
    out = outp.reshape(-1, P, H, WP)[:, :, :, :W].astype(np.float32)
    return np.ascontiguousarray(out), res


def kernel(**inputs) -> np.ndarray:
    out, _ = _run(inputs, trace=False)
    return out


# revision 7
# speedup vs baseline: 1.0304x; 1.0304x over previous
"""VQ-codebook 3x3 conv (nn_CConv) on 8 Trainium2 NeuronCores.

Sharding: data-parallel over the batch (16 images -> 2 per core); the small
codebook-derived weights / scales / bias are replicated to every core.
Host-side work is layout/dtype prep only: batch split, zero-padding of x to
114x114 with an f16 cast, reshape/transpose of the index and scale matrices,
the codebook row gather (pure indexing), and stripping the two junk columns
from the padded f16 output.

Per-core device program (one NEFF, SPMD over 8 cores):
  - weight build (on device): multiply the gathered codebook rows by
    scales (pre-rounded through f16 on host = dequant emulation) * cut;
    weights stored k-major so each of the 9 taps is a contiguous
    [128(in), 128(out)] fp16 stationary block.  Weight DMAs ride the
    gpsimd HWDGE queue so they overlap the first input-image load.
  - conv: each padded image lives whole in SBUF ([128, 114*114] f16,
    loaded in 4 contiguous chunked DMAs on the sync queue); the 3x3 conv
    is 9 accumulating PE matmuls over shifted views of the flattened
    padded image, fp16 in / fp32 PSUM accumulate, 25 PSUM tiles of
    <=512 positions per image.  Outputs are computed in padded q space
    (q = h*114 + w); junk columns w in {112,113} are kept in the f16
    DRAM output and stripped on host, so both input and output DMAs are
    fully contiguous per partition.
  - PSUM is evacuated with a fused per-partition bias add on the vector
    engine straight to f16; output DMAs ride the scalar HWDGE queue in
    row-chunks as soon as their tiles are evacuated.
  - 8 dummy warm-up matmuls run during the prologue so the PE HAM clock
    gate reaches 2.4 GHz before the real matmuls start.
"""
import sys
import types
from contextlib import ExitStack

import numpy as np

import concourse.tile as tile
from concourse import bacc, mybir


def _ensure_axon_hooks_module():
    """This image's antenv package lacks axon_hooks; bass_utils imports it
    when tracing is requested (e.g. BASS_TRACE=1). Provide a no-op shim."""
    try:
        import antenv

        if "antenv.axon_hooks" not in sys.modules and not hasattr(
            antenv, "axon_hooks"
        ):
            mod = types.ModuleType("antenv.axon_hooks")
            holder = [None]
            mod.set_axon_ntff_profile_hook = lambda h: holder.__setitem__(0, h)
            mod.get_axon_ntff_profile_hook = lambda: holder[0]
            antenv.axon_hooks = mod
            sys.modules["antenv.axon_hooks"] = mod
    except Exception:
        pass


_ensure_axon_hooks_module()

from concourse import bass_utils  # noqa: E402

P = 128
H = W = 112
WP = 114
NPIX = WP * WP          # padded image pixels per channel (114*114 = 12996)
NQ = H * WP             # output q-space length incl. junk cols (112*114 = 12768)
IMGS = 2
N_CORES = 8

f32 = mybir.dt.float32
f16 = mybir.dt.float16

IN_CHUNKS = [(0, 8), (8, 32), (32, 60), (60, 88), (88, 114)]  # padded input rows
OUT_CHUNKS = [(0, 28), (28, 56), (56, 84), (84, 112)]      # output rows
OUT_CHUNKS_LAST = [(0, 28), (28, 56), (56, 84), (84, 106), (106, 112)]

_CACHE = {}


def _q_tiles():
    full, r = divmod(NQ, 512)
    return [512] * full + ([r] if r else [])


def _build():
    nc = bacc.Bacc("TRN2", target_bir_lowering=False, debug=False)

    x_t = nc.dram_tensor("x", [IMGS, P, NPIX], f16, kind="ExternalInput")
    bias_t = nc.dram_tensor("bias", [P, 1], f32, kind="ExternalInput")
    wmm_t = nc.dram_tensor("wmm", [P, P * 9], f16, kind="ExternalInput")
    out_t = nc.dram_tensor("out", [IMGS, P, NQ], f16, kind="ExternalOutput")

    with tile.TileContext(nc) as tc, ExitStack() as ctx:
        wb = ctx.enter_context(tc.tile_pool(name="wb", bufs=1))
        xp = ctx.enter_context(tc.tile_pool(name="xp", bufs=2))
        op = ctx.enter_context(tc.tile_pool(name="op", bufs=2))
        ps = ctx.enter_context(tc.tile_pool(name="ps", bufs=7, space="PSUM"))

        # no dummy PE warmup: the first ~3.4us of real matmuls run at the
        # HAM cold clock either way; cold real work beats warm-after-dummies

        # ---- weights: host-built w_mm[i, k, o] (k-major f16 taps) ----
        w_mm = wb.tile([P, 9 * P], f16, tag="w_mm")
        nc.sync.dma_start(w_mm[:], wmm_t.ap())
        bias_s = wb.tile([P, 1], f32, tag="bias")
        nc.sync.dma_start(bias_s[:], bias_t.ap())
        w_k_view = w_mm[:].rearrange("p (k o) -> p k o", o=P)

        # ---- conv, whole image resident in SBUF ----
        for img in range(IMGS):
            xpad = xp.tile([P, NPIX + 2], f16, tag="xpad")
            # last 2 junk positions read 2 elements past the padded image
            nc.gpsimd.memset(xpad[:, NPIX:NPIX + 2], 0.0)
            for r0, r1 in IN_CHUNKS:
                nc.sync.dma_start(
                    xpad[:, r0 * WP:r1 * WP], x_t.ap()[img, :, r0 * WP:r1 * WP]
                )

            oimg = op.tile([P, NQ], f16, tag="oimg")
            ochunks = list(OUT_CHUNKS_LAST if img == IMGS - 1 else OUT_CHUNKS)
            q0 = 0
            for n in _q_tiles():
                pst = ps.tile([P, 512], f32, tag="pst")
                for k in range(9):
                    dh, dw = divmod(k, 3)
                    off = q0 + dh * WP + dw
                    nc.tensor.matmul(
                        pst[:, :n],
                        w_k_view[:, k, :],
                        xpad[:, off:off + n],
                        start=(k == 0),
                        stop=(k == 8),
                    )
                nc.vector.tensor_scalar_add(
                    oimg[:, q0:q0 + n], pst[:, :n], bias_s[:, 0:1]
                )
                q0 += n
                while ochunks and q0 >= ochunks[0][1] * WP:
                    r0, r1 = ochunks.pop(0)
                    nc.scalar.dma_start(
                        out_t.ap()[img, :, r0 * WP:r1 * WP],
                        oimg[:, r0 * WP:r1 * WP],
                    )
            assert not ochunks

    nc.compile()
    return nc


def _make_in_maps(inputs):
    x = np.asarray(inputs["x"], dtype=np.float32)
    nimg = x.shape[0]
    xpad = np.zeros((nimg, P, WP, WP), dtype=np.float16)
    xpad[:, :, 1:1 + H, 1:1 + W] = x.astype(np.float16)
    xpad = xpad.reshape(nimg, P, NPIX)

    cent = np.asarray(inputs["centroids"], dtype=np.float32).reshape(512, 9)
    idxT = np.asarray(inputs["idx"]).reshape(P, P).T          # [i, o]
    # fp16 round-trip of scales (dequant emulation), * cut
    scalesT = (
        np.asarray(inputs["scales"], dtype=np.float32).reshape(P, P).T
        .astype(np.float16).astype(np.float32)
    )
    cutT = np.asarray(inputs["cut"], dtype=np.float32).reshape(P, P).T
    bias = np.ascontiguousarray(
        np.asarray(inputs["bias"], dtype=np.float32).reshape(P, 1)
    )
    # w_mm[i, k, o] = w_raw[i, o, k] * scales_q[i, o] * cut[i, o], f16 taps
    wraw = cent[idxT].reshape(P, P, 9)                        # [i, o, k]
    wmm = np.ascontiguousarray(
        (wraw * (scalesT * cutT)[:, :, None])
        .transpose(0, 2, 1).reshape(P, P * 9).astype(np.float16)
    )

    base = {"bias": bias, "wmm": wmm}
    maps = []
    for c in range(N_CORES):
        m = dict(base)
        m["x"] = np.ascontiguousarray(xpad[IMGS * c:IMGS * (c + 1)])
        maps.append(m)
    return maps


def _get_nc():
    if "nc" not in _CACHE:
        _CACHE["nc"] = _build()
    return _CACHE["nc"]


def _run(inputs, trace=False):
    nc = _get_nc()
    in_maps = _make_in_maps(inputs)
    res = bass_utils.run_bass_kernel_spmd(
        nc, in_maps, core_ids=list(range(N_CORES)), trace=trace
    )
    outp = np.concatenate(
        [res.results[c]["out"] for c in range(N_CORES)], axis=0
    )
    out = outp.reshape(-1, P, H, WP)[:, :, :, :W].astype(np.float32)
    return np.ascontiguousarray(out), res


def kernel(**inputs) -> np.ndarray:
    out, _ = _run(inputs, trace=False)
    return out
